# revision 11
# baseline (speedup 1.0000x reference)
"""Causal self-attention with RoPE on 8 Trainium2 NeuronCores.

Sharding: core c = 4*b + g handles batch b (of 2) and head group g (4 of 16
heads). Each core computes q/k/v projections for its heads, head-local causal
softmax attention, and a partial output projection (Wp columns of its heads);
the host sums the 4 partials per batch and adds bp.

Layout strategy (per core):
  xsb   [128,512] x32 : xT resident in SBUF, tile (i, qtr); loaded once over
                  two DMA queues, shared by the q/k and v projections.
  qT/kT [128,T]  : per j-tile (2 heads each), partition = head dim, f32r.
  v''   [128,260] bf16 x16 : natural layout per t-tile; 65 cols/head =
                  [v_head | ones] so the AV matmul's row 64 accumulates the
                  softmax denominator for free.
  S^T   [s, t]   : scores transposed; exp(0.125*S) on ACT -> P^T in bf16.
  AV    [65,512] chunks: yT_unnorm (rows 0..63) + r (row 64); copied out as
                  soon as each 512-t chunk's accumulation closes, so two
                  heads pipeline through two PSUM chunk buffers.
  norm  : 1/r broadcast to [128, t] via a K=4 indicator matmul; 1 DVE mult.
  proj  : outT[e, t] partial = WpT_g.T @ yT  (bias added on host).

Phase order: q/k + RoPE (DMA streams split: x on sync+scalar queues, tables
on gpsimd), v-projection from resident x (pure PE; covers the last RoPE
drain), attention (ACT exp is the long pole; PSUM chunking keeps both PE and
ACT dense), then norm+proj with norm emission running ahead of proj.
"""

import sys

for _p in ("/opt/trn_rl_repo",):
    if _p not in sys.path:
        sys.path.append(_p)

import numpy as np
import ml_dtypes
from contextlib import ExitStack

import concourse.bacc as bacc
import concourse.tile as tile
from concourse import mybir
from concourse.bass_utils import run_bass_kernel_spmd

F32 = mybir.dt.float32
F32R = mybir.dt.float32r
BF16 = mybir.dt.bfloat16
EXP = mybir.ActivationFunctionType.Exp

B, T, C = 2, 2048, 1024
H, D = 16, 64
HG = 4                 # heads per core
JG = HG * D            # 256 j-columns per core
VW = HG * 65           # v'' width (64 dims + ones col per head)
NKT = C // 128         # 8 contraction tiles
NTT = T // 128         # 16 t-tiles / s-tiles
NC4 = T // 512         # 4 512-chunks
SCALE = 1.0 / np.sqrt(D)

_NC_CACHE = None


def build_bass(zero_bias=False):
    nc = bacc.Bacc()

    xT = nc.declare_dram_parameter("xT", [C, T], F32, isOutput=False)
    wqT = nc.declare_dram_parameter("wqT", [C, JG], F32, isOutput=False)
    wkT = nc.declare_dram_parameter("wkT", [C, JG], F32, isOutput=False)
    wvT = nc.declare_dram_parameter("wvT", [C, JG], F32, isOutput=False)
    wpT = nc.declare_dram_parameter("wpT", [JG, C], F32, isOutput=False)
    cosT = nc.declare_dram_parameter("cosT", [128, T], F32, isOutput=False)
    ssT = nc.declare_dram_parameter("ssT", [128, T], F32, isOutput=False)
    bmask = nc.declare_dram_parameter("bmask", [128, 384], BF16, isOutput=False)
    ind4 = nc.declare_dram_parameter("ind4", [4, JG], F32, isOutput=False)
    vones = nc.declare_dram_parameter("vones", [128, HG], BF16, isOutput=False)
    if not zero_bias:
        bq_r = nc.declare_dram_parameter("bq_r", [1, JG], F32, isOutput=False)
        bk_r = nc.declare_dram_parameter("bk_r", [1, JG], F32, isOutput=False)
        bv_r = nc.declare_dram_parameter("bv_r", [1, JG], F32, isOutput=False)
        ones_r = nc.declare_dram_parameter("ones_r", [1, 512], F32, isOutput=False)

    outT = nc.declare_dram_parameter("outT", [C, T], F32, isOutput=True)

    with (
        tile.TileContext(nc) as tc,
        ExitStack() as ctx,
        nc.allow_low_precision(reason="f32r/bf16 matmul pipeline"),
    ):
        consts = ctx.enter_context(tc.tile_pool(name="consts", bufs=1))

        # ---- queue plan ------------------------------------------------
        # sync:   x q0, q1 (all i), x q2/q3 even i; later outT (even et),
        #         r-row staging DMAs during attention
        # scalar: wq, wk per-ctile; x q2/q3 odd i
        # gpsimd: cos, ss, vones, wv per-ctile, bmask, ind4, wp;
        #         later outT (odd et)
        def wtile(name):
            return consts.tile([128, NKT * JG], F32R, tag=name, name=name)

        wq_sb, wk_sb, wv_sb = wtile("wq"), wtile("wk"), wtile("wv")

        def load_w_ctile(eng, t, dram, i):
            eng.dma_start(
                t[:, i * JG : (i + 1) * JG],
                dram[128 * i : 128 * (i + 1), :].bitcast(F32R),
            )

        # x resident: 32 tiles [128,512], tag x{i}_{q}
        xsb = [
            [
                consts.tile([128, 512], F32R, tag=f"x{i}_{q}", name=f"x{i}_{q}")
                for q in range(4)
            ]
            for i in range(NKT)
        ]

        def load_x(eng, i, q):
            eng.dma_start(
                xsb[i][q][:],
                xT[128 * i : 128 * (i + 1), 512 * q : 512 * (q + 1)].bitcast(F32R),
            )

        for i in range(NKT):
            load_w_ctile(nc.scalar, wq_sb, wqT, i)
            load_w_ctile(nc.scalar, wk_sb, wkT, i)
        for q in range(4):
            for i in range(NKT):
                if q >= 2 and i % 2 == 1:
                    load_x(nc.scalar, i, q)
                else:
                    load_x(nc.sync, i, q)

        cos_sb = consts.tile([128, T], F32, tag="cos", name="cos")
        ss_sb = consts.tile([128, T], F32, tag="ss", name="ss")
        nc.gpsimd.dma_start(cos_sb[:], cosT[:])
        nc.gpsimd.dma_start(ss_sb[:], ssT[:])
        vones_sb = consts.tile([128, HG], BF16, tag="vones", name="vones")
        nc.gpsimd.dma_start(vones_sb[:], vones[:])
        for i in range(NKT):
            load_w_ctile(nc.gpsimd, wv_sb, wvT, i)
        bm_sb = consts.tile([128, 384], BF16, tag="bmask", name="bmask")
        nc.gpsimd.dma_start(bm_sb[:], bmask[:])
        ind4_sb = consts.tile([4, JG], F32R, tag="ind4", name="ind4")
        nc.gpsimd.dma_start(ind4_sb[:], ind4[:].bitcast(F32R))
        wp_sb = [None, None]
        for jt in range(2):
            wp_sb[jt] = consts.tile([128, C], F32R, tag=f"wp{jt}", name=f"wp{jt}")
            nc.gpsimd.dma_start(
                wp_sb[jt][:], wpT[128 * jt : 128 * (jt + 1), :].bitcast(F32R)
            )
        if not zero_bias:
            def load_const(name, dram, shape):
                t = consts.tile(shape, F32R, tag=name, name=name)
                nc.gpsimd.dma_start(t[:], dram[:].bitcast(F32R))
                return t

            bq_sb = load_const("bq", bq_r, [1, JG])
            bk_sb = load_const("bk", bk_r, [1, JG])
            bv_sb = load_const("bv", bv_r, [1, JG])
            ones_sb = load_const("ones", ones_r, [1, 512])

        qkv_sb = ctx.enter_context(tc.tile_pool(name="qkv", bufs=1))
        qT_sb = [qkv_sb.tile([128, T], F32R, tag=f"qT{j}", name=f"qT{j}") for j in range(2)]
        kT_sb = [qkv_sb.tile([128, T], F32R, tag=f"kT{j}", name=f"kT{j}") for j in range(2)]
        yT_sb = [qkv_sb.tile([128, T], F32R, tag=f"yT{j}", name=f"yT{j}") for j in range(2)]
        v_sb = [qkv_sb.tile([128, VW], BF16, tag=f"v{s}", name=f"v{s}") for s in range(NTT)]
        rr4_sb = qkv_sb.tile([4, T], F32R, tag="rr4", name="rr4")
        rscr_sb = qkv_sb.tile([1, T], F32, tag="rscr", name="rscr")

        # ---- q/k phase: T-quarters, 4 slots (q-j0,k-j0,q-j1,k-j1) ------
        # RoPE pipelines under the next quarter's matmuls (psum 2 quarters
        # deep). One DVE copy drains PSUM; the rotate_half partition swap
        # rides free SBUF->SBUF DMA bandwidth; 3 all-SBUF DVE ops combine.
        # ACT does nothing here, so it is exclusively exp's engine later.
        rope_pool = ctx.enter_context(tc.tile_pool(name="rope", bufs=2))
        rope_q = [nc.gpsimd, nc.scalar, nc.sync]
        with tc.tile_pool(name="pqk", bufs=8, space="PSUM") as pqk:
            for qtr in range(4):
                tlo = 512 * qtr
                ps4 = [
                    pqk.tile([128, 512], F32, tag="pqk", name="pqk")
                    for _ in range(4)
                ]
                for i in range(NKT):
                    for sl, (jt, w_sb) in enumerate(
                        ((0, wq_sb), (0, wk_sb), (1, wq_sb), (1, wk_sb))
                    ):
                        nc.tensor.matmul(
                            ps4[sl][:],
                            w_sb[:, i * JG + 128 * jt : i * JG + 128 * (jt + 1)],
                            xsb[i][qtr][:],
                            start=(i == 0),
                            stop=(zero_bias and i == NKT - 1),
                        )
                for sl, (jt, bias_key, dst) in enumerate(
                    (
                        (0, "bq", qT_sb),
                        (0, "bk", kT_sb),
                        (1, "bq", qT_sb),
                        (1, "bk", kT_sb),
                    )
                ):
                    p = ps4[sl]
                    if not zero_bias:
                        b_sb = bq_sb if bias_key == "bq" else bk_sb
                        nc.tensor.matmul(
                            p[:],
                            b_sb[:, 128 * jt : 128 * (jt + 1)],
                            ones_sb[:, :],
                            start=False,
                            stop=True,
                        )
                    # RoPE: dst = p*cos + rotate_half(p)*ss
                    out = dst[jt][:, tlo : tlo + 512]
                    qtmp = rope_pool.tile([128, 512], F32, tag="qtmp", name="qtmp")
                    nc.vector.tensor_copy(qtmp[:], p[:])
                    rin = rope_pool.tile([128, 512], F32, tag="rin", name="rin")
                    dq = rope_q[(4 * qtr + sl) % 3]
                    for h0 in (0, 64):
                        a_, b_, c_ = h0, h0 + 32, h0 + 64
                        dq.dma_start(rin[a_:b_, :], qtmp[b_:c_, :])
                        dq.dma_start(rin[b_:c_, :], qtmp[a_:b_, :])
                    nc.vector.tensor_mul(out, qtmp[:], cos_sb[:, tlo : tlo + 512])
                    rot = rope_pool.tile([128, 512], F32R, tag="rot", name="rot")
                    nc.vector.tensor_mul(rot[:], rin[:], ss_sb[:, tlo : tlo + 512])
                    nc.vector.tensor_add(out, out, rot[:])

        # ---- v phase: pure PE from resident x; covers the q3 RoPE drain
        with tc.tile_pool(name="pv", bufs=2, space="PSUM") as pv:
            for qtr in range(4):
                pvt = pv.tile([128, 4 * JG], F32, tag="pv", name="pv")
                for tt4 in range(4):
                    sl = pvt[:, JG * tt4 : JG * (tt4 + 1)]
                    for i in range(NKT):
                        nc.tensor.matmul(
                            sl,
                            xsb[i][qtr][:, 128 * tt4 : 128 * (tt4 + 1)],
                            wv_sb[:, i * JG : (i + 1) * JG],
                            start=(i == 0),
                            stop=(zero_bias and i == NKT - 1),
                        )
                    if not zero_bias:
                        nc.tensor.matmul(
                            sl, ones_sb[:, :128], bv_sb[:, :], start=False, stop=True
                        )
                for tt4 in range(4):
                    tt = 4 * qtr + tt4
                    vv = v_sb[tt][:].rearrange("p (h w) -> p h w", h=HG)
                    nc.vector.tensor_copy(
                        vv[:, :, 0:64],
                        pvt[:, JG * tt4 : JG * (tt4 + 1)].rearrange(
                            "p (h w) -> p h w", h=HG
                        ),
                    )
                    nc.gpsimd.tensor_copy(
                        vv[:, :, 64:65], vones_sb[:].rearrange("p (h w) -> p h w", w=1)
                    )

        # ---- attention: two T-half passes (pass A needs only half-0 rope)
        with (
            tc.tile_pool(name="pst", bufs=3, space="PSUM") as pst,
            tc.tile_pool(name="pav", bufs=2, space="PSUM") as pav,
            tc.tile_pool(name="ppt", bufs=3) as ppt,
        ):
            for lo in (0, 1024):
                nk = (lo + 1024) // 128  # s-tiles in this pass
                for hl in range(HG):
                    jt, m = hl // 2, hl % 2
                    qh = qT_sb[jt][64 * m : 64 * (m + 1), :]
                    kh = kT_sb[jt][64 * m : 64 * (m + 1), :]
                    # per-512-chunk AV accumulators; chunk c covers
                    # t in [lo+512c, lo+512c+512)
                    p_av = [
                        pav.tile([65, 512], F32, tag="pav", name="pav")
                        for _ in range(2)
                    ]
                    ppts = [None] * nk

                    def emit_st(k, ppts=ppts, qh=qh, kh=kh):
                        t0 = 128 * k
                        a = max(t0 - lo, 0)
                        estart = min(a, 256) if a < 512 else 512 + min(a - 512, 256)
                        p_st = pst.tile([128, 1024], F32, tag="pst", name="pst")
                        for s5 in range(2):
                            slo = lo + 512 * s5
                            if slo + 512 <= t0:
                                continue
                            sa = min(max(t0 - slo, 0), 256)
                            nc.tensor.matmul(
                                p_st[:, 512 * s5 + sa : 512 * (s5 + 1)],
                                kh[:, 128 * k : 128 * (k + 1)],
                                qh[:, slo + sa : slo + 512],
                                start=True,
                                stop=True,
                            )
                        pt = ppt.tile([128, 1024], BF16, tag="ppt", name="ppt")
                        nc.scalar.activation(
                            pt[:, estart:], p_st[:, estart:], EXP, scale=float(SCALE)
                        )
                        if t0 >= lo:
                            w = a + 128 - estart
                            nc.vector.tensor_mul(
                                pt[:, estart : a + 128],
                                pt[:, estart : a + 128],
                                bm_sb[:, 384 - w : 384],
                            )
                        ppts[k] = pt

                    def emit_av(k, ppts=ppts, p_av=p_av, hl=hl, nk=nk):
                        t0 = 128 * k
                        vh = v_sb[k][:, 65 * hl : 65 * (hl + 1)]
                        for c in range(2):
                            slo = lo + 512 * c
                            if slo + 512 <= t0:
                                continue
                            sa = min(max(t0 - slo, 0), 256)
                            nc.tensor.matmul(
                                p_av[c][:, sa:512],
                                vh,
                                ppts[k][:, 512 * c + sa : 512 * (c + 1)],
                                start=(k == 0),
                                stop=(k == min((slo + 512) // 128 - 1, nk - 1)),
                            )

                    def emit_copies(k, p_av=p_av, jt=jt, m=m, lo=lo, nk=nk):
                        # chunk c's accumulation closes at k == stop_k(c);
                        # drain it immediately so the PSUM buffer recycles
                        for c in range(2):
                            if k != min((lo + 512 * c + 512) // 128 - 1, nk - 1):
                                continue
                            clo = lo + 512 * c
                            nc.vector.tensor_copy(
                                yT_sb[jt][64 * m : 64 * (m + 1), clo : clo + 512],
                                p_av[c][0:64, :],
                            )
                            nc.vector.tensor_copy(
                                rscr_sb[0:1, clo : clo + 512].bitcast(F32R),
                                p_av[c][64:65, :],
                            )
                            nc.sync.dma_start(
                                rr4_sb[2 * jt + m : 2 * jt + m + 1, clo : clo + 512],
                                rscr_sb[0:1, clo : clo + 512].bitcast(F32R),
                            )

                    for k in range(nk):
                        emit_st(k)
                        if k > 0:
                            emit_av(k - 1)
                            emit_copies(k - 1)
                    emit_av(nk - 1)
                    emit_copies(nk - 1)

        # ---- norm (yT /= r per head) + output projection ---------------
        # norm emission runs one c4 ahead of proj so the DVE chain hides
        # under the previous chunk's projection matmuls.
        with (
            tc.tile_pool(name="prb", bufs=2, space="PSUM") as prb,
            tc.tile_pool(name="rb", bufs=2) as rb_pool,
            tc.tile_pool(name="pp", bufs=4, space="PSUM") as pp,
            tc.tile_pool(name="ostage", bufs=2) as ostage,
        ):
            def emit_norm(c4):
                for jt in range(2):
                    pn = prb.tile([128, 512], F32, tag="prb", name="prb")
                    nc.tensor.matmul(
                        pn[:],
                        ind4_sb[:, 128 * jt : 128 * (jt + 1)],
                        rr4_sb[:, 512 * c4 : 512 * (c4 + 1)],
                        start=True,
                        stop=True,
                    )
                    rb = rb_pool.tile([128, 512], F32, tag="rb", name="rb")
                    nc.vector.reciprocal_approx_fast(out=rb[:], in_=pn[:])
                    sl = yT_sb[jt][:, 512 * c4 : 512 * (c4 + 1)]
                    nc.vector.tensor_mul(sl, sl, rb[:].bitcast(F32R))

            def emit_proj(c4):
                for et in range(8):
                    p = pp.tile([128, 512], F32, tag="pp", name="pp")
                    for jt in range(2):
                        nc.tensor.matmul(
                            p[:],
                            wp_sb[jt][:, 128 * et : 128 * (et + 1)],
                            yT_sb[jt][:, 512 * c4 : 512 * (c4 + 1)],
                            start=(jt == 0),
                            stop=(jt == 1),
                        )
                    o = ostage.tile([128, 512], F32, tag="ostage", name="ostage")
                    if et % 2 == 0:
                        nc.scalar.copy(o[:], p[:])
                    else:
                        nc.vector.tensor_copy(o[:], p[:])
                    (nc.sync if et % 2 == 0 else nc.gpsimd).dma_start(
                        outT[128 * et : 128 * (et + 1), 512 * c4 : 512 * (c4 + 1)],
                        o[:],
                    )

            emit_norm(0)
            emit_norm(1)
            emit_proj(0)
            emit_norm(2)
            emit_proj(1)
            emit_norm(3)
            emit_proj(2)
            emit_proj(3)

    nc.finalize()
    return nc


def _rope_tables():
    inv_freq = 1.0 / (10000.0 ** (np.arange(0, D, 2, dtype=np.float32) / D))
    t = np.arange(T, dtype=np.float32)
    freqs = t[:, None] * inv_freq[None, :]              # [T, 32]
    emb = np.concatenate([freqs, freqs], axis=1)        # [T, 64]
    cos = np.cos(emb).astype(np.float32).T              # [64, T]
    sin = np.sin(emb).astype(np.float32).T              # [64, T]
    # rotate_half signs at destination rows: rot[d<32] = -q[d+32]*sin[d]
    ss = np.concatenate([-sin[:32], sin[32:]], axis=0)
    cosT = np.concatenate([cos, cos], axis=0)           # [128, T] (2 heads)
    ssT = np.concatenate([ss, ss], axis=0)              # [128, T]
    return np.ascontiguousarray(cosT), np.ascontiguousarray(ssT)


def _host_inputs(x, Wq, bq, Wk, bk, Wv, bv, Wp, bp, zero_bias):
    cosT, ssT = _rope_tables()
    s = np.arange(128)[:, None]
    u = np.arange(384)[None, :]
    bmask = ((u - 256) >= s).astype(ml_dtypes.bfloat16)
    ind4 = np.zeros((4, JG), np.float32)
    for j in range(JG):
        ind4[2 * (j // 128) + (j % 128) // 64, j] = 1.0
    vones = np.ones((128, HG), ml_dtypes.bfloat16)

    maps = []
    for b in range(B):
        for g in range(4):
            J = slice(g * JG, (g + 1) * JG)
            m = {
                "xT": np.ascontiguousarray(x[b].T),
                "wqT": np.ascontiguousarray(Wq[J, :].T),
                "wkT": np.ascontiguousarray(Wk[J, :].T),
                "wvT": np.ascontiguousarray(Wv[J, :].T),
                "wpT": np.ascontiguousarray(Wp[:, J].T),
                "cosT": cosT,
                "ssT": ssT,
                "bmask": bmask,
                "ind4": ind4,
                "vones": vones,
            }
            if not zero_bias:
                m["bq_r"] = bq[None, J].astype(np.float32)
                m["bk_r"] = bk[None, J].astype(np.float32)
                m["bv_r"] = bv[None, J].astype(np.float32)
                m["ones_r"] = np.ones((1, 512), np.float32)
            maps.append(m)
    return maps


def kernel(x, Wq, bq, Wk, bk, Wv, bv, Wp, bp, _trace=False):
    global _NC_CACHE
    x, Wq, bq, Wk, bk, Wv, bv, Wp, bp = (
        np.asarray(a, np.float32) for a in (x, Wq, bq, Wk, bk, Wv, bv, Wp, bp)
    )
    zb = not (np.any(bq) or np.any(bk) or np.any(bv))
    if _NC_CACHE is None or _NC_CACHE[1] != zb:
        _NC_CACHE = (build_bass(zero_bias=zb), zb)
    maps = _host_inputs(x, Wq, bq, Wk, bk, Wv, bv, Wp, bp, zb)
    res = run_bass_kernel_spmd(_NC_CACHE[0], maps, list(range(8)), trace=_trace)
    out = np.empty((B, T, C), np.float32)
    for b in range(B):
        acc = res.results[4 * b]["outT"].copy()
        for g in range(1, 4):
            acc += res.results[4 * b + g]["outT"]
        out[b] = acc.T + bp[None, :]
    if _trace:
        return out, res
    return out


# revision 12
# speedup vs baseline: 1.6644x; 1.6644x over previous
"""Causal self-attention with RoPE on 8 Trainium2 NeuronCores.

Sharding: core c = 4*b + g handles batch b (of 2) and head group g (4 of 16
heads). Each core computes q/k/v projections for its heads, head-local causal
softmax attention, and a partial output projection (Wp columns of its heads);
the host sums the 4 partials per batch and adds bp.

Layout strategy (per core):
  xsb   [128,512] x32 : xT resident in SBUF, tile (i, qtr); loaded once over
                  two DMA queues, shared by the q/k and v projections.
  qT/kT [128,T]  : per j-tile (2 heads each), partition = head dim, f32r.
  v''   [128,260] bf16 x16 : natural layout per t-tile; 65 cols/head =
                  [v_head | ones] so the AV matmul's row 64 accumulates the
                  softmax denominator for free.
  S^T   [s, t]   : scores transposed; exp(0.125*S) on ACT -> P^T in bf16.
  AV    [65,512] chunks: yT_unnorm (rows 0..63) + r (row 64); copied out as
                  soon as each 512-t chunk's accumulation closes, so two
                  heads pipeline through two PSUM chunk buffers.
  norm  : 1/r broadcast to [128, t] via a K=4 indicator matmul; 1 DVE mult.
  proj  : outT[e, t] partial = WpT_g.T @ yT  (bias added on host).

Phase order: q/k + RoPE (DMA streams split: x on sync+scalar queues, tables
on gpsimd), v-projection from resident x (pure PE; covers the last RoPE
drain), attention (ACT exp is the long pole; PSUM chunking keeps both PE and
ACT dense), then norm+proj with norm emission running ahead of proj.
"""

import sys

for _p in ("/opt/trn_rl_repo",):
    if _p not in sys.path:
        sys.path.append(_p)

import numpy as np
import ml_dtypes
from contextlib import ExitStack

import concourse.bacc as bacc
import concourse.tile as tile
from concourse import mybir
from concourse.bass_utils import run_bass_kernel_spmd

F32 = mybir.dt.float32
F32R = mybir.dt.float32r
BF16 = mybir.dt.bfloat16
EXP = mybir.ActivationFunctionType.Exp

B, T, C = 2, 2048, 1024
H, D = 16, 64
HG = 4                 # heads per core
JG = HG * D            # 256 j-columns per core
VW = HG * 65           # v'' width (64 dims + ones col per head)
NKT = C // 128         # 8 contraction tiles
NTT = T // 128         # 16 t-tiles / s-tiles
NC4 = T // 512         # 4 512-chunks
SCALE = 1.0 / np.sqrt(D)

_NC_CACHE = None


def build_bass(zero_bias=False):
    nc = bacc.Bacc()

    xT = nc.declare_dram_parameter("xT", [C, T], F32, isOutput=False)
    wqT = nc.declare_dram_parameter("wqT", [C, JG], F32, isOutput=False)
    wkT = nc.declare_dram_parameter("wkT", [C, JG], F32, isOutput=False)
    wvT = nc.declare_dram_parameter("wvT", [C, JG], F32, isOutput=False)
    wpT = nc.declare_dram_parameter("wpT", [JG, C], F32, isOutput=False)
    cosT = nc.declare_dram_parameter("cosT", [128, T], BF16, isOutput=False)
    ssT = nc.declare_dram_parameter("ssT", [128, T], BF16, isOutput=False)
    bmask = nc.declare_dram_parameter("bmask", [128, 384], BF16, isOutput=False)
    ind4 = nc.declare_dram_parameter("ind4", [4, JG], F32, isOutput=False)
    vones = nc.declare_dram_parameter("vones", [128, HG], BF16, isOutput=False)
    if not zero_bias:
        bq_r = nc.declare_dram_parameter("bq_r", [1, JG], F32, isOutput=False)
        bk_r = nc.declare_dram_parameter("bk_r", [1, JG], F32, isOutput=False)
        bv_r = nc.declare_dram_parameter("bv_r", [1, JG], F32, isOutput=False)
        ones_r = nc.declare_dram_parameter("ones_r", [1, 512], F32, isOutput=False)

    outT = nc.declare_dram_parameter("outT", [C, T], F32, isOutput=True)

    with (
        tile.TileContext(nc) as tc,
        ExitStack() as ctx,
        nc.allow_low_precision(reason="f32r/bf16 matmul pipeline"),
    ):
        consts = ctx.enter_context(tc.tile_pool(name="consts", bufs=1))

        # ---- queue plan ------------------------------------------------
        # sync:   x q0, q1 (all i), x q2/q3 even i; later outT (even et),
        #         r-row staging DMAs during attention
        # scalar: wq, wk per-ctile; x q2/q3 odd i
        # gpsimd: cos, ss, vones, wv per-ctile, bmask, ind4, wp;
        #         later outT (odd et)
        def wtile(name):
            return consts.tile([128, NKT * JG], F32R, tag=name, name=name)

        wq_sb, wk_sb, wv_sb = wtile("wq"), wtile("wk"), wtile("wv")

        def load_w_ctile(eng, t, dram, i):
            eng.dma_start(
                t[:, i * JG : (i + 1) * JG],
                dram[128 * i : 128 * (i + 1), :].bitcast(F32R),
            )

        # x resident: 32 tiles [128,512], tag x{i}_{q}
        xsb = [
            [
                consts.tile([128, 512], F32R, tag=f"x{i}_{q}", name=f"x{i}_{q}")
                for q in range(4)
            ]
            for i in range(NKT)
        ]

        def load_x(eng, i, q):
            eng.dma_start(
                xsb[i][q][:],
                xT[128 * i : 128 * (i + 1), 512 * q : 512 * (q + 1)].bitcast(F32R),
            )

        for i in range(NKT):
            load_w_ctile(nc.scalar, wq_sb, wqT, i)
            load_w_ctile(nc.scalar, wk_sb, wkT, i)
        for q in range(4):
            for i in range(NKT):
                if q >= 2 and i % 2 == 1:
                    load_x(nc.scalar, i, q)
                else:
                    load_x(nc.sync, i, q)

        cos_sb = consts.tile([128, T], BF16, tag="cos", name="cos")
        ss_sb = consts.tile([128, T], BF16, tag="ss", name="ss")
        nc.gpsimd.dma_start(cos_sb[:], cosT[:])
        nc.gpsimd.dma_start(ss_sb[:], ssT[:])
        vones_sb = consts.tile([128, HG], BF16, tag="vones", name="vones")
        nc.gpsimd.dma_start(vones_sb[:], vones[:])
        for i in range(NKT):
            load_w_ctile(nc.gpsimd, wv_sb, wvT, i)
        bm_sb = consts.tile([128, 384], BF16, tag="bmask", name="bmask")
        nc.gpsimd.dma_start(bm_sb[:], bmask[:])
        ind4_sb = consts.tile([4, JG], F32R, tag="ind4", name="ind4")
        nc.gpsimd.dma_start(ind4_sb[:], ind4[:].bitcast(F32R))
        wp_sb = [None, None]
        for jt in range(2):
            wp_sb[jt] = consts.tile([128, C], F32R, tag=f"wp{jt}", name=f"wp{jt}")
            nc.gpsimd.dma_start(
                wp_sb[jt][:], wpT[128 * jt : 128 * (jt + 1), :].bitcast(F32R)
            )
        if not zero_bias:
            def load_const(name, dram, shape):
                t = consts.tile(shape, F32R, tag=name, name=name)
                nc.gpsimd.dma_start(t[:], dram[:].bitcast(F32R))
                return t

            bq_sb = load_const("bq", bq_r, [1, JG])
            bk_sb = load_const("bk", bk_r, [1, JG])
            bv_sb = load_const("bv", bv_r, [1, JG])
            ones_sb = load_const("ones", ones_r, [1, 512])

        qkv_sb = ctx.enter_context(tc.tile_pool(name="qkv", bufs=1))
        qT_sb = [qkv_sb.tile([128, T], BF16, tag=f"qT{j}", name=f"qT{j}") for j in range(2)]
        kT_sb = [qkv_sb.tile([128, T], BF16, tag=f"kT{j}", name=f"kT{j}") for j in range(2)]
        yT_sb = [qkv_sb.tile([128, T], F32R, tag=f"yT{j}", name=f"yT{j}") for j in range(2)]
        v_sb = [qkv_sb.tile([128, VW], BF16, tag=f"v{s}", name=f"v{s}") for s in range(NTT)]
        rr4_sb = qkv_sb.tile([4, T], F32R, tag="rr4", name="rr4")
        rscr_sb = qkv_sb.tile([1, T], F32, tag="rscr", name="rscr")

        # ---- q/k phase: T-quarters, 4 slots (q-j0,k-j0,q-j1,k-j1) ------
        # RoPE pipelines under the next quarter's matmuls (psum 2 quarters
        # deep). One DVE copy drains PSUM; the rotate_half partition swap
        # rides free SBUF->SBUF DMA bandwidth; 3 all-SBUF DVE ops combine.
        # ACT does nothing here, so it is exclusively exp's engine later.
        rope_pool = ctx.enter_context(tc.tile_pool(name="rope", bufs=3))
        rope_q = [nc.gpsimd, nc.scalar]
        with tc.tile_pool(name="pqk", bufs=8, space="PSUM") as pqk:
            for qtr in range(4):
                tlo = 512 * qtr
                ps4 = [
                    pqk.tile([128, 512], F32, tag="pqk", name="pqk")
                    for _ in range(4)
                ]
                for i in range(NKT):
                    for sl, (jt, w_sb) in enumerate(
                        ((0, wq_sb), (0, wk_sb), (1, wq_sb), (1, wk_sb))
                    ):
                        nc.tensor.matmul(
                            ps4[sl][:],
                            w_sb[:, i * JG + 128 * jt : i * JG + 128 * (jt + 1)],
                            xsb[i][qtr][:],
                            start=(i == 0),
                            stop=(zero_bias and i == NKT - 1),
                        )
                for sl, (jt, bias_key, dst) in enumerate(
                    (
                        (0, "bq", qT_sb),
                        (0, "bk", kT_sb),
                        (1, "bq", qT_sb),
                        (1, "bk", kT_sb),
                    )
                ):
                    p = ps4[sl]
                    if not zero_bias:
                        b_sb = bq_sb if bias_key == "bq" else bk_sb
                        nc.tensor.matmul(
                            p[:],
                            b_sb[:, 128 * jt : 128 * (jt + 1)],
                            ones_sb[:, :],
                            start=False,
                            stop=True,
                        )
                    # RoPE: dst = p*cos + rotate_half(p)*ss
                    out = dst[jt][:, tlo : tlo + 512]
                    qtmp = rope_pool.tile([128, 512], BF16, tag="qtmp", name="qtmp")
                    nc.scalar.copy(qtmp[:], p[:])
                    rin = rope_pool.tile([128, 512], BF16, tag="rin", name="rin")
                    dq = rope_q[(4 * qtr + sl) % 2]
                    for h0 in (0, 64):
                        a_, b_, c_ = h0, h0 + 32, h0 + 64
                        dq.dma_start(rin[a_:b_, :], qtmp[b_:c_, :])
                        dq.dma_start(rin[b_:c_, :], qtmp[a_:b_, :])
                    nc.vector.tensor_mul(out, qtmp[:], cos_sb[:, tlo : tlo + 512])
                    rot = rope_pool.tile([128, 512], BF16, tag="rot", name="rot")
                    nc.vector.tensor_mul(rot[:], rin[:], ss_sb[:, tlo : tlo + 512])
                    nc.vector.tensor_add(out, out, rot[:])

        # ---- v phase: pure PE from resident x; covers the q3 RoPE drain
        with tc.tile_pool(name="pv", bufs=2, space="PSUM") as pv:
            for qtr in range(4):
                pvt = pv.tile([128, 4 * JG], F32, tag="pv", name="pv")
                for tt4 in range(4):
                    sl = pvt[:, JG * tt4 : JG * (tt4 + 1)]
                    for i in range(NKT):
                        nc.tensor.matmul(
                            sl,
                            xsb[i][qtr][:, 128 * tt4 : 128 * (tt4 + 1)],
                            wv_sb[:, i * JG : (i + 1) * JG],
                            start=(i == 0),
                            stop=(zero_bias and i == NKT - 1),
                        )
                    if not zero_bias:
                        nc.tensor.matmul(
                            sl, ones_sb[:, :128], bv_sb[:, :], start=False, stop=True
                        )
                for tt4 in range(4):
                    tt = 4 * qtr + tt4
                    vv = v_sb[tt][:].rearrange("p (h w) -> p h w", h=HG)
                    nc.vector.tensor_copy(
                        vv[:, :, 0:64],
                        pvt[:, JG * tt4 : JG * (tt4 + 1)].rearrange(
                            "p (h w) -> p h w", h=HG
                        ),
                    )
                    nc.gpsimd.tensor_copy(
                        vv[:, :, 64:65], vones_sb[:].rearrange("p (h w) -> p h w", w=1)
                    )

        # ---- attention: two T-half passes (pass A needs only half-0 rope)
        with (
            tc.tile_pool(name="pst", bufs=3, space="PSUM") as pst,
            tc.tile_pool(name="pav", bufs=2, space="PSUM") as pav,
            tc.tile_pool(name="ppt", bufs=4) as ppt,
        ):
            for lo in (0, 1024):
                nk = (lo + 1024) // 128  # s-tiles in this pass
                for hl in range(HG):
                    jt, m = hl // 2, hl % 2
                    qh = qT_sb[jt][64 * m : 64 * (m + 1), :]
                    kh = kT_sb[jt][64 * m : 64 * (m + 1), :]
                    # per-512-chunk AV accumulators; chunk c covers
                    # t in [lo+512c, lo+512c+512)
                    p_av = [
                        pav.tile([65, 512], F32, tag="pav", name="pav")
                        for _ in range(2)
                    ]
                    ppts = [None] * nk

                    def emit_st(k, ppts=ppts, qh=qh, kh=kh):
                        t0 = 128 * k
                        a = max(t0 - lo, 0)
                        estart = min(a, 256) if a < 512 else 512 + min(a - 512, 256)
                        p_st = pst.tile([128, 1024], F32, tag="pst", name="pst")
                        for s5 in range(2):
                            slo = lo + 512 * s5
                            if slo + 512 <= t0:
                                continue
                            sa = min(max(t0 - slo, 0), 256)
                            nc.tensor.matmul(
                                p_st[:, 512 * s5 + sa : 512 * (s5 + 1)],
                                kh[:, 128 * k : 128 * (k + 1)],
                                qh[:, slo + sa : slo + 512],
                                start=True,
                                stop=True,
                            )
                        pt = ppt.tile([128, 1024], BF16, tag="ppt", name="ppt")
                        nc.scalar.activation(
                            pt[:, estart:], p_st[:, estart:], EXP, scale=float(SCALE)
                        )
                        if t0 >= lo:
                            w = a + 128 - estart
                            nc.vector.tensor_mul(
                                pt[:, estart : a + 128],
                                pt[:, estart : a + 128],
                                bm_sb[:, 384 - w : 384],
                            )
                        ppts[k] = pt

                    def emit_av(k, ppts=ppts, p_av=p_av, hl=hl, nk=nk):
                        t0 = 128 * k
                        vh = v_sb[k][:, 65 * hl : 65 * (hl + 1)]
                        for c in range(2):
                            slo = lo + 512 * c
                            if slo + 512 <= t0:
                                continue
                            sa = min(max(t0 - slo, 0), 256)
                            nc.tensor.matmul(
                                p_av[c][:, sa:512],
                                vh,
                                ppts[k][:, 512 * c + sa : 512 * (c + 1)],
                                start=(k == 0),
                                stop=(k == min((slo + 512) // 128 - 1, nk - 1)),
                            )

                    def emit_copies(k, p_av=p_av, jt=jt, m=m, lo=lo, nk=nk):
                        # chunk c's accumulation closes at k == stop_k(c);
                        # drain it immediately so the PSUM buffer recycles
                        for c in range(2):
                            if k != min((lo + 512 * c + 512) // 128 - 1, nk - 1):
                                continue
                            clo = lo + 512 * c
                            nc.vector.tensor_copy(
                                yT_sb[jt][64 * m : 64 * (m + 1), clo : clo + 512],
                                p_av[c][0:64, :],
                            )
                            nc.vector.tensor_copy(
                                rscr_sb[0:1, clo : clo + 512].bitcast(F32R),
                                p_av[c][64:65, :],
                            )
                            nc.sync.dma_start(
                                rr4_sb[2 * jt + m : 2 * jt + m + 1, clo : clo + 512],
                                rscr_sb[0:1, clo : clo + 512].bitcast(F32R),
                            )

                    for k in range(nk):
                        emit_st(k)
                        if k > 0:
                            emit_av(k - 1)
                            emit_copies(k - 1)
                    emit_av(nk - 1)
                    emit_copies(nk - 1)

        # ---- norm (yT /= r per head) + output projection ---------------
        # norm emission runs one c4 ahead of proj so the DVE chain hides
        # under the previous chunk's projection matmuls.
        with (
            tc.tile_pool(name="prb", bufs=2, space="PSUM") as prb,
            tc.tile_pool(name="rb", bufs=2) as rb_pool,
            tc.tile_pool(name="pp", bufs=4, space="PSUM") as pp,
            tc.tile_pool(name="ostage", bufs=4) as ostage,
        ):
            def emit_norm(c4):
                for jt in range(2):
                    pn = prb.tile([128, 512], F32, tag="prb", name="prb")
                    nc.tensor.matmul(
                        pn[:],
                        ind4_sb[:, 128 * jt : 128 * (jt + 1)],
                        rr4_sb[:, 512 * c4 : 512 * (c4 + 1)],
                        start=True,
                        stop=True,
                    )
                    rb = rb_pool.tile([128, 512], F32, tag="rb", name="rb")
                    nc.vector.reciprocal_approx_fast(out=rb[:], in_=pn[:])
                    sl = yT_sb[jt][:, 512 * c4 : 512 * (c4 + 1)]
                    nc.vector.tensor_mul(sl, sl, rb[:].bitcast(F32R))

            def emit_proj(c4):
                for et in range(8):
                    p = pp.tile([128, 512], F32, tag="pp", name="pp")
                    for jt in range(2):
                        nc.tensor.matmul(
                            p[:],
                            wp_sb[jt][:, 128 * et : 128 * (et + 1)],
                            yT_sb[jt][:, 512 * c4 : 512 * (c4 + 1)],
                            start=(jt == 0),
                            stop=(jt == 1),
                        )
                    o = ostage.tile([128, 512], F32, tag="ostage", name="ostage")
                    if et % 2 == 0:
                        nc.scalar.copy(o[:], p[:])
                    else:
                        nc.vector.tensor_copy(o[:], p[:])
                    (nc.sync if et % 2 == 0 else nc.gpsimd).dma_start(
                        outT[128 * et : 128 * (et + 1), 512 * c4 : 512 * (c4 + 1)],
                        o[:],
                    )

            emit_norm(0)
            emit_norm(1)
            emit_proj(0)
            emit_norm(2)
            emit_proj(1)
            emit_norm(3)
            emit_proj(2)
            emit_proj(3)

    nc.finalize()
    return nc


def _rope_tables():
    inv_freq = 1.0 / (10000.0 ** (np.arange(0, D, 2, dtype=np.float32) / D))
    t = np.arange(T, dtype=np.float32)
    freqs = t[:, None] * inv_freq[None, :]              # [T, 32]
    emb = np.concatenate([freqs, freqs], axis=1)        # [T, 64]
    cos = np.cos(emb).astype(np.float32).T              # [64, T]
    sin = np.sin(emb).astype(np.float32).T              # [64, T]
    # rotate_half signs at destination rows: rot[d<32] = -q[d+32]*sin[d]
    ss = np.concatenate([-sin[:32], sin[32:]], axis=0)
    cosT = np.concatenate([cos, cos], axis=0)           # [128, T] (2 heads)
    ssT = np.concatenate([ss, ss], axis=0)              # [128, T]
    return (
        np.ascontiguousarray(cosT).astype(ml_dtypes.bfloat16),
        np.ascontiguousarray(ssT).astype(ml_dtypes.bfloat16),
    )


def _host_inputs(x, Wq, bq, Wk, bk, Wv, bv, Wp, bp, zero_bias):
    cosT, ssT = _rope_tables()
    s = np.arange(128)[:, None]
    u = np.arange(384)[None, :]
    bmask = ((u - 256) >= s).astype(ml_dtypes.bfloat16)
    ind4 = np.zeros((4, JG), np.float32)
    for j in range(JG):
        ind4[2 * (j // 128) + (j % 128) // 64, j] = 1.0
    vones = np.ones((128, HG), ml_dtypes.bfloat16)

    maps = []
    for b in range(B):
        for g in range(4):
            J = slice(g * JG, (g + 1) * JG)
            m = {
                "xT": np.ascontiguousarray(x[b].T),
                "wqT": np.ascontiguousarray(Wq[J, :].T),
                "wkT": np.ascontiguousarray(Wk[J, :].T),
                "wvT": np.ascontiguousarray(Wv[J, :].T),
                "wpT": np.ascontiguousarray(Wp[:, J].T),
                "cosT": cosT,
                "ssT": ssT,
                "bmask": bmask,
                "ind4": ind4,
                "vones": vones,
            }
            if not zero_bias:
                m["bq_r"] = bq[None, J].astype(np.float32)
                m["bk_r"] = bk[None, J].astype(np.float32)
                m["bv_r"] = bv[None, J].astype(np.float32)
                m["ones_r"] = np.ones((1, 512), np.float32)
            maps.append(m)
    return maps


def kernel(x, Wq, bq, Wk, bk, Wv, bv, Wp, bp, _trace=False):
    global _NC_CACHE
    x, Wq, bq, Wk, bk, Wv, bv, Wp, bp = (
        np.asarray(a, np.float32) for a in (x, Wq, bq, Wk, bk, Wv, bv, Wp, bp)
    )
    zb = not (np.any(bq) or np.any(bk) or np.any(bv))
    if _NC_CACHE is None or _NC_CACHE[1] != zb:
        _NC_CACHE = (build_bass(zero_bias=zb), zb)
    maps = _host_inputs(x, Wq, bq, Wk, bk, Wv, bv, Wp, bp, zb)
    res = run_bass_kernel_spmd(_NC_CACHE[0], maps, list(range(8)), trace=_trace)
    out = np.empty((B, T, C), np.float32)
    for b in range(B):
        acc = res.results[4 * b]["outT"].copy()
        for g in range(1, 4):
            acc += res.results[4 * b + g]["outT"]
        out[b] = acc.T + bp[None, :]
    if _trace:
        return out, res
    return out
